# revision 30
# baseline (speedup 1.0000x reference)
"""Trainium2 Bass kernel for nn_ConvAttention_34600256537137.

Math notes (validated against the reference):
  qkv = 1x1conv(x, w1)+b1 -> Q,K,V;  score = conv5x5(Q_s)+conv5x5(K_t)+b2;
  attn = softmax_t(score);  out = einsum(attn, V).
  Softmax over t is shift-invariant, so the Q-half of the score (constant in
  t), b2, and the K-path bias all cancel.  The computation collapses to:
    weff[ci,dy,dx] = sum_c w1K[c,ci] * w2K[c,dy,dx]        (host, tiny)
    sK[b,t,h,w]    = conv5x5_reflect(x[b,:,:,:,t], weff)
    e = exp(sK);  den = sum_t e
    out[b,o,h,w,s] = (sum_{ci,t} w1V[o,ci] * e * x) / den + b1V[o]
  (s-independent; the S-broadcast and +b1V happen on host)

Sharding: 8 cores = (b in {0,1}) x (4 chunks of 8 rows of H).

Perf structure (v6):
  - all matmul operands are bf16: 1 cycle/row on PE and half the HBM bytes
    for the two big loads (slab, xt).  rhs streams stay contiguous.
  - every DMA costs ~1.5us fixed + 0.9us semaphore propagation, so the
    design minimizes serial DMA hops: one packed constants load, 4 slab
    chunks, 4 td stores, 5 gathers (the first two overlap the conv tail),
    SBUF->SBUF bounces for the tiny e/rcp layout changes.
  - conv: T[tap,(w,t)] = weff^T @ slab per half-row on PE; PSUM->SBUF
    copies (vector+scalar alternating) write s_T[tap, r, t, w] bf16;
    five per-dy 3-dim-AP gathers build s_R[(h,t), tap, w]; per-dy partial
    tap reduces pipeline behind the gathers.
  - spatial partition packing is (h,t): h=p//16, t=p%16; e replication
    over ci8 and the den broadcast are indicator matmuls on the PE; the
    1/den normalization fuses into the output PSUM read.
"""

import sys

if "/opt/trn_rl_repo" not in sys.path:
    sys.path.insert(0, "/opt/trn_rl_repo")

import numpy as np

B, C, H, W, S = 2, 64, 32, 32, 16
KS, PAD = 5, 2
NCORES = 8
ROWS = H // 4            # output rows per core (8)
SLAB_R = ROWS + 2 * PAD  # 12
SLAB_W = W + 2 * PAD     # 36
NTAP = KS * KS           # 25
HW = ROWS * W            # 256 output positions
HREST = SLAB_W // 2      # 18
NPOS = SLAB_R * S * SLAB_W  # 6912

# packed constants layout (columns in s_const [128, NCONST])
OFF_WEFF, OFF_HSEL, OFF_IREP, OFF_W1VR = 0, 25, 33, 161
NCONST = 161 + 8 * C

_MODULE = None


def _build_module():
    import concourse.bacc as bacc
    import concourse.bass as bass
    import concourse.tile as tile
    from concourse import mybir

    f32 = mybir.dt.float32
    bf16 = mybir.dt.bfloat16
    AF = mybir.ActivationFunctionType
    ALU = mybir.AluOpType
    nc = bacc.Bacc("TRN2", target_bir_lowering=False, debug=False, num_devices=NCORES)

    slab_d = nc.dram_tensor("slab", [C, SLAB_R, SLAB_W, S], bf16, kind="ExternalInput")
    xt_d = nc.dram_tensor("xt", [128, 8, HW], bf16, kind="ExternalInput")
    const_d = nc.dram_tensor("cst", [128, NCONST], bf16, kind="ExternalInput")
    o_d = nc.dram_tensor("o", [C, HW], f32, kind="ExternalOutput")

    tda_d = nc.dram_tensor("tda", [NTAP, 9, S, SLAB_W], bf16)     # T rows 0-8
    tdb_d = nc.dram_tensor("tdb", [NTAP, 3, S, SLAB_W], bf16)     # T rows 9-11
    rd_d = nc.dram_tensor("rd", [HW], f32)                        # 1/den, flat
    ed_d = nc.dram_tensor("ed", [S, ROWS, W], bf16)               # e, t-major

    with tile.TileContext(nc) as tc:
        with tc.tile_pool(name="sb", bufs=1) as sb, tc.tile_pool(
            name="ps", bufs=3, space="PSUM"
        ) as ps, tc.tile_pool(name="pso", bufs=1, space="PSUM") as pso:
            # --- loads: weff tiny on gpsimd first; 6x 2-row slab chunks ride
            # all three queues in consumption order; constants + split xt
            # follow late ---
            s_weff = sb.tile([C, NTAP], bf16)
            nc.gpsimd.dma_start(s_weff, const_d.ap()[:C, OFF_WEFF : OFF_WEFF + NTAP])
            slab_t = []
            for ch in range(6):
                st = sb.tile([C, 2, SLAB_W, S], bf16, tag=f"slab{ch}")
                slab_t.append(st)
                (nc.sync, nc.scalar, nc.gpsimd)[ch % 3].dma_start(
                    st, slab_d.ap()[:, 2 * ch : 2 * ch + 2, :, :]
                )
            s_const = sb.tile([128, NCONST], bf16)
            nc.gpsimd.dma_start(s_const, const_d.ap())
            s_hsel = s_const[:, OFF_HSEL : OFF_HSEL + ROWS]
            s_xt = sb.tile([128, 8, HW], bf16)

            # --- phase 1: T = weff^T @ slab, half-row chunks, contiguous
            # rhs stream (w,t); the PSUM->SBUF copy applies the transpose ---
            s_T = sb.tile([NTAP, SLAB_R, S, SLAB_W], bf16)
            s_R = sb.tile([128, NTAP, W], bf16)

            def gather(dy, h0, h1, eng):
                # reads td rows dy+h0 .. dy+h1-1 from td_a (rows 0-8) or
                # td_b (rows 9-11); flat offset for (h,t,dx,w):
                #   (5*dy+dx)*rows*576 + (h+dy-base)*576 + t*36 + (w+dx)
                td, base, rows = (tda_d, 0, 9) if dy + h1 <= 9 else (tdb_d, 9, 3)
                rstr = rows * S * SLAB_W
                src = bass.AP(
                    tensor=td.ap().tensor,
                    offset=5 * dy * rstr + (h0 + dy - base) * S * SLAB_W,
                    ap=[
                        [SLAB_W, (h1 - h0) * S],  # (h,t) partitions
                        [rstr + 1, KS],           # dx
                        [1, W],                   # w
                    ],
                )
                eng.dma_start(s_R[S * h0 : S * h1, KS * dy : KS * (dy + 1), :], src)

            for row in range(SLAB_R):
                p2 = ps.tile([NTAP, 2, 2 * S, S], f32, tag="pt")
                for half in range(2):
                    nc.tensor.matmul(
                        p2[:, half, :HREST, :],
                        s_weff,
                        slab_t[row // 2][
                            :, row % 2, half * HREST : (half + 1) * HREST, :
                        ],
                        start=True,
                        stop=True,
                    )
                dst = s_T[:, row, :, :]
                src = p2[:, :, :HREST, :].transpose([0, 3, 1, 2])
                if row % 2 == 0:
                    nc.vector.tensor_copy(dst, src)
                else:
                    nc.scalar.copy(dst, src)
                if row == 1:      # rows 0-1 stored early
                    nc.sync.dma_start(
                        tda_d.ap()[:, 0:2, :, :], s_T[:, 0:2, :, :]
                    )
                elif row == 8:    # rows 2-8: all five big gathers unblock
                    nc.sync.dma_start(
                        tda_d.ap()[:, 2:9, :, :], s_T[:, 2:9, :, :]
                    )
                    gather(0, 0, 8, nc.sync)
                    gather(1, 0, 8, nc.gpsimd)
                    gather(2, 0, 7, nc.sync)
                    gather(3, 0, 6, nc.gpsimd)
                    gather(4, 0, 5, nc.scalar)
                elif row == 11:   # rows 9-11 -> td_b
                    nc.scalar.dma_start(tdb_d.ap(), s_T[:, 9:12, :, :])
            gather(2, 7, 8, nc.scalar)
            gather(3, 6, 8, nc.sync)
            gather(4, 5, 8, nc.scalar)
            # split xt halves ride behind the gathers on two queues
            nc.gpsimd.dma_start(s_xt[:, 0:4, :], xt_d.ap()[:, 0:4, :])
            nc.scalar.dma_start(s_xt[:, 4:8, :], xt_d.ap()[:, 4:8, :])

            # --- per-dy partial tap reduces, ordered by gather arrival ---
            s_pd = {}
            for dy in (0, 1, 2, 4, 3):
                p = sb.tile([128, W], f32, tag=f"pd{dy}")
                nc.vector.tensor_reduce(
                    p,
                    s_R[:, KS * dy : KS * (dy + 1), :].transpose([0, 2, 1]),
                    axis=mybir.AxisListType.X,
                    op=ALU.add,
                )
                s_pd[dy] = p
                if dy == 1:
                    s_a01 = sb.tile([128, W], f32)
                    nc.vector.tensor_tensor(s_a01, s_pd[0], s_pd[1], op=ALU.add)
                if dy == 4:
                    s_a04 = sb.tile([128, W], f32)
                    nc.vector.tensor_tensor(s_a04, s_pd[2], s_pd[4], op=ALU.add)
                    nc.vector.tensor_tensor(s_a04, s_a01, s_a04, op=ALU.add)
            s_sk = sb.tile([128, W], f32)
            nc.vector.tensor_tensor(s_sk, s_a04, s_pd[3], op=ALU.add)

            # --- e = exp(sK); SBUF->SBUF bounce to [t, (h,w)]; den path
            # (den -> rcp -> stride-0 broadcast) runs in parallel ---
            s_e = sb.tile([128, W], bf16)
            nc.scalar.activation(s_e, s_sk, AF.Exp)
            nc.sync.dma_start(
                bass.AP(
                    tensor=ed_d.ap().tensor,
                    offset=0,
                    ap=[[W, ROWS], [ROWS * W, S], [1, W]],
                ),
                s_e,
            )
            p_den = pso.tile([ROWS, W], f32, tag="den")
            nc.tensor.matmul(p_den, s_hsel, s_e, start=True, stop=True)
            s_rcp = sb.tile([ROWS, W], f32)
            nc.vector.reciprocal(s_rcp, p_den)
            nc.scalar.dma_start(rd_d.ap(), s_rcp)
            s_rcpb = sb.tile([C, HW], f32)
            nc.scalar.dma_start(
                s_rcpb,
                bass.AP(tensor=rd_d.ap().tensor, offset=0, ap=[[0, C], [1, HW]]),
            )

            # --- replicate e over ci8 via a stride-0 broadcast DRAM read,
            # on the same queue as the e store ---
            s_eb = sb.tile([128, HW], bf16)
            nc.sync.dma_start(
                s_eb,
                bass.AP(
                    tensor=ed_d.ap().tensor,
                    offset=0,
                    ap=[[0, 8], [HW, S], [1, HW]],
                ),
            )

            # --- V path: xa = xt * e (two halves overlap the V matmuls);
            # contract (ci8,t) on PE; normalize on the PSUM read ---
            s_xa = sb.tile([128, 8, HW], bf16)
            p_o = pso.tile([C, HW], f32, tag="out")
            for halfg in range(2):
                nc.vector.tensor_tensor(
                    s_xa[:, 4 * halfg : 4 * halfg + 4, :],
                    s_xt[:, 4 * halfg : 4 * halfg + 4, :],
                    s_eb.unsqueeze(1).broadcast_to((128, 4, HW)),
                    op=ALU.mult,
                )
                for g in range(4 * halfg, 4 * halfg + 4):
                    nc.tensor.matmul(
                        p_o,
                        s_const[:, OFF_W1VR + C * g : OFF_W1VR + C * (g + 1)],
                        s_xa[:, g, :],
                        start=(g == 0),
                        stop=(g == 7),
                    )
            s_o = sb.tile([C, HW], f32)
            nc.vector.tensor_tensor(s_o, p_o, s_rcpb, op=ALU.mult)
            nc.sync.dma_start(o_d.ap(), s_o)

    nc.compile()
    return nc


def _get_module():
    global _MODULE
    if _MODULE is None:
        _MODULE = _build_module()
    return _MODULE


def make_host_inputs(x, w1, b1, w2, b2):
    """Host-side precompute: folded weights + per-core reflect-padded slices."""
    import ml_dtypes

    bf16 = ml_dtypes.bfloat16
    x = np.ascontiguousarray(np.asarray(x, np.float32))
    w1 = np.asarray(w1, np.float32)
    w2 = np.asarray(w2, np.float32)

    w1K = w1[C : 2 * C, :, 0, 0]          # [c, ci]
    w2K = w2[0, C : 2 * C]                # [c, 5, 5]
    weff = np.einsum("ci,cyx->iyx", w1K, w2K).reshape(C, NTAP)
    w1V = w1[2 * C :, :, 0, 0]            # [co, ci]

    # w1vr[(ci8,t), g, co] = w1V[co, 8g+ci8]
    tmp = w1V.T.reshape(8, 8, C)                      # (g, ci8, co)
    w1vr = (
        np.broadcast_to(tmp[:, :, None, :], (8, 8, S, C))
        .transpose(1, 2, 0, 3)
        .reshape(128, 8 * C)
    )

    # spatial partition packing is (h,t): p = h*16 + t
    hsel = np.zeros((128, ROWS), np.float32)
    for h in range(ROWS):
        hsel[h * S : (h + 1) * S, h] = 1.0
    irep = np.zeros((S, 128), np.float32)             # [t', (ci8,t)]
    for t in range(S):
        irep[t, t::S] = 1.0

    const = np.zeros((128, NCONST), np.float32)
    const[:C, OFF_WEFF : OFF_WEFF + NTAP] = weff
    const[:, OFF_HSEL : OFF_HSEL + ROWS] = hsel
    const[:S, OFF_IREP : OFF_IREP + 128] = irep
    const[:, OFF_W1VR:] = w1vr
    const = const.astype(bf16)

    in_maps = []
    for core in range(NCORES):
        b, hc = divmod(core, 4)
        h0 = ROWS * hc
        xp = np.pad(x[b], ((0, 0), (PAD, PAD), (PAD, PAD), (0, 0)), mode="reflect")
        slab = np.ascontiguousarray(xp[:, h0 : h0 + SLAB_R, :, :]).astype(bf16)
        xs = x[b][:, h0 : h0 + ROWS, :, :]            # [ci, h, w, t]
        xt = np.ascontiguousarray(
            xs.reshape(8, 8, ROWS, W, S)
            .transpose(1, 4, 0, 2, 3)
            .reshape(128, 8, HW)
        ).astype(bf16)
        in_maps.append({"slab": slab, "xt": xt, "cst": const})
    return in_maps


def assemble_output(results, b1):
    b1V = np.asarray(b1, np.float32)[2 * C :]
    out = np.empty((B, C, H, W, S), np.float32)
    for core in range(NCORES):
        b, hc = divmod(core, 4)
        h0 = ROWS * hc
        o = results[core]["o"].reshape(C, ROWS, W).astype(np.float32)
        out[b, :, h0 : h0 + ROWS, :, :] = (
            o[:, :, :, None] + b1V[:, None, None, None]
        )
    return out


def kernel(x, w1, b1, w2, b2):
    from concourse.bass_utils import run_bass_kernel_spmd

    nc = _get_module()
    in_maps = make_host_inputs(x, w1, b1, w2, b2)
    res = run_bass_kernel_spmd(nc, in_maps, core_ids=list(range(NCORES)))
    return assemble_output(res.results, b1)


# revision 34
# speedup vs baseline: 1.0414x; 1.0414x over previous
"""Trainium2 Bass kernel for nn_ConvAttention_34600256537137.

Math notes (validated against the reference):
  qkv = 1x1conv(x, w1)+b1 -> Q,K,V;  score = conv5x5(Q_s)+conv5x5(K_t)+b2;
  attn = softmax_t(score);  out = einsum(attn, V).
  Softmax over t is shift-invariant, so the Q-half of the score (constant in
  t), b2, and the K-path bias all cancel.  The computation collapses to:
    weff[ci,dy,dx] = sum_c w1K[c,ci] * w2K[c,dy,dx]        (host, tiny)
    sK[b,t,h,w]    = conv5x5_reflect(x[b,:,:,:,t], weff)
    e = exp(sK);  den = sum_t e
    out[b,o,h,w,s] = (sum_{ci,t} w1V[o,ci] * e * x) / den + b1V[o]
  (s-independent; the S-broadcast and +b1V happen on host)

Sharding: 8 cores = (b in {0,1}) x (4 chunks of 8 rows of H).

Perf structure (v6):
  - all matmul operands are bf16: 1 cycle/row on PE and half the HBM bytes
    for the two big loads (slab, xt).  rhs streams stay contiguous.
  - every DMA costs ~1.5us fixed + 0.9us semaphore propagation, so the
    design minimizes serial DMA hops: one packed constants load, 4 slab
    chunks, 4 td stores, 5 gathers (the first two overlap the conv tail),
    SBUF->SBUF bounces for the tiny e/rcp layout changes.
  - conv: T[tap,(w,t)] = weff^T @ slab per half-row on PE; PSUM->SBUF
    copies (vector+scalar alternating) write s_T[tap, r, t, w] bf16;
    five per-dy 3-dim-AP gathers build s_R[(h,t), tap, w]; per-dy partial
    tap reduces pipeline behind the gathers.
  - spatial partition packing is (h,t): h=p//16, t=p%16; e replication
    over ci8 and the den broadcast are indicator matmuls on the PE; the
    1/den normalization fuses into the output PSUM read.
"""

import sys

if "/opt/trn_rl_repo" not in sys.path:
    sys.path.insert(0, "/opt/trn_rl_repo")

import numpy as np

B, C, H, W, S = 2, 64, 32, 32, 16
KS, PAD = 5, 2
NCORES = 8
ROWS = H // 4            # output rows per core (8)
SLAB_R = ROWS + 2 * PAD  # 12
SLAB_W = W + 2 * PAD     # 36
NTAP = KS * KS           # 25
HW = ROWS * W            # 256 output positions
HREST = SLAB_W // 2      # 18
NPOS = SLAB_R * S * SLAB_W  # 6912

# packed constants layout (columns in s_const [128, NCONST])
OFF_WEFF, OFF_HSEL, OFF_IREP, OFF_W1VR = 0, 25, 33, 161
NCONST = 161 + 8 * C

_MODULE = None


def _build_module():
    import concourse.bacc as bacc
    import concourse.bass as bass
    import concourse.tile as tile
    from concourse import mybir

    f32 = mybir.dt.float32
    bf16 = mybir.dt.bfloat16
    AF = mybir.ActivationFunctionType
    ALU = mybir.AluOpType
    nc = bacc.Bacc("TRN2", target_bir_lowering=False, debug=False, num_devices=NCORES)

    slab_d = nc.dram_tensor("slab", [C, SLAB_R, SLAB_W, S], bf16, kind="ExternalInput")
    xt_d = nc.dram_tensor("xt", [128, 8, HW], bf16, kind="ExternalInput")
    const_d = nc.dram_tensor("cst", [128, NCONST], bf16, kind="ExternalInput")
    o_d = nc.dram_tensor("o", [C, HW], f32, kind="ExternalOutput")

    td_d = nc.dram_tensor("td", [NTAP, SLAB_R, S, SLAB_W], bf16)  # T, tap-major
    rd_d = nc.dram_tensor("rd", [HW], f32)                        # 1/den, flat
    ed_d = nc.dram_tensor("ed", [S, ROWS, W], bf16)               # e, t-major

    with tile.TileContext(nc) as tc:
        with tc.tile_pool(name="sb", bufs=1) as sb, tc.tile_pool(
            name="ps", bufs=3, space="PSUM"
        ) as ps, tc.tile_pool(name="pso", bufs=1, space="PSUM") as pso:
            # --- loads: weff tiny on gpsimd first; 6x 2-row slab chunks ride
            # all three queues in consumption order; constants + split xt
            # follow late ---
            s_weff = sb.tile([C, NTAP], bf16)
            nc.gpsimd.dma_start(s_weff, const_d.ap()[:C, OFF_WEFF : OFF_WEFF + NTAP])
            slab_t = []
            for ch in range(6):
                st = sb.tile([C, 2, SLAB_W, S], bf16, tag=f"slab{ch}")
                slab_t.append(st)
                (nc.sync, nc.scalar, nc.gpsimd)[ch % 3].dma_start(
                    st, slab_d.ap()[:, 2 * ch : 2 * ch + 2, :, :]
                )
            s_const = sb.tile([128, NCONST], bf16)
            nc.gpsimd.dma_start(s_const, const_d.ap())
            s_hsel = s_const[:, OFF_HSEL : OFF_HSEL + ROWS]
            s_xt = sb.tile([128, 8, HW], bf16)

            # --- phase 1: T = weff^T @ slab, half-row chunks, contiguous
            # rhs stream (w,t); the PSUM->SBUF copy applies the transpose ---
            s_T = sb.tile([NTAP, SLAB_R, S, SLAB_W], bf16)
            s_R = sb.tile([128, NTAP, W], bf16)

            def gather(dy, eng):
                # td flat offset for (h,t,dy,dx,w):
                #   (5*dy+dx)*6912 + (h+dy)*576 + t*36 + (w+dx)
                src = bass.AP(
                    tensor=td_d.ap().tensor,
                    offset=dy * (5 * NPOS + S * SLAB_W),
                    ap=[
                        [SLAB_W, 128],       # (h,t) partitions
                        [NPOS + 1, KS],      # dx
                        [1, W],              # w
                    ],
                )
                eng.dma_start(s_R[:, KS * dy : KS * (dy + 1), :], src)

            for row in range(SLAB_R):
                p2 = ps.tile([NTAP, 2, 2 * S, S], f32, tag="pt")
                for half in range(2):
                    nc.tensor.matmul(
                        p2[:, half, :HREST, :],
                        s_weff,
                        slab_t[row // 2][
                            :, row % 2, half * HREST : (half + 1) * HREST, :
                        ],
                        start=True,
                        stop=True,
                    )
                dst = s_T[:, row, :, :]
                src = p2[:, :, :HREST, :].transpose([0, 3, 1, 2])
                if row % 2 == 0:
                    nc.vector.tensor_copy(dst, src)
                else:
                    nc.scalar.copy(dst, src)
                if row in (2, 5, 8, 11):  # pipelined 3-row stores, all sync
                    gr = row // 3
                    nc.sync.dma_start(
                        td_d.ap()[:, 3 * gr : 3 * gr + 3, :, :],
                        s_T[:, 3 * gr : 3 * gr + 3, :, :],
                    )
            gather(0, nc.sync)
            gather(3, nc.scalar)
            gather(1, nc.gpsimd)
            gather(2, nc.sync)
            gather(4, nc.scalar)
            # split xt halves ride behind the gathers on two queues
            nc.gpsimd.dma_start(s_xt[:, 0:4, :], xt_d.ap()[:, 0:4, :])
            nc.scalar.dma_start(s_xt[:, 4:8, :], xt_d.ap()[:, 4:8, :])

            # --- per-dy partial tap reduces, ordered by gather arrival ---
            s_pd = {}
            s_acc = sb.tile([128, W], f32)
            for i, dy in enumerate((0, 3, 1, 2, 4)):
                p = sb.tile([128, W], f32, tag=f"pd{dy}")
                nc.vector.tensor_reduce(
                    p,
                    s_R[:, KS * dy : KS * (dy + 1), :].transpose([0, 2, 1]),
                    axis=mybir.AxisListType.X,
                    op=ALU.add,
                )
                s_pd[dy] = p
                if i == 1:
                    nc.vector.tensor_tensor(s_acc, s_pd[0], p, op=ALU.add)
                elif i > 1:
                    nc.vector.tensor_tensor(s_acc, s_acc, p, op=ALU.add)
            s_sk = s_acc

            # --- e = exp(sK); SBUF->SBUF bounce to [t, (h,w)]; den path
            # (den -> rcp -> stride-0 broadcast) runs in parallel ---
            s_e = sb.tile([128, W], bf16)
            nc.scalar.activation(s_e, s_sk, AF.Exp)
            nc.sync.dma_start(
                bass.AP(
                    tensor=ed_d.ap().tensor,
                    offset=0,
                    ap=[[W, ROWS], [ROWS * W, S], [1, W]],
                ),
                s_e,
            )
            p_den = pso.tile([ROWS, W], f32, tag="den")
            nc.tensor.matmul(p_den, s_hsel, s_e, start=True, stop=True)
            s_rcp = sb.tile([ROWS, W], f32)
            nc.vector.reciprocal(s_rcp, p_den)
            nc.scalar.dma_start(rd_d.ap(), s_rcp)
            s_rcpb = sb.tile([C, HW], f32)
            nc.scalar.dma_start(
                s_rcpb,
                bass.AP(tensor=rd_d.ap().tensor, offset=0, ap=[[0, C], [1, HW]]),
            )

            # --- replicate e over ci8 via a stride-0 broadcast DRAM read,
            # on the same queue as the e store ---
            s_eb = sb.tile([128, HW], bf16)
            nc.sync.dma_start(
                s_eb,
                bass.AP(
                    tensor=ed_d.ap().tensor,
                    offset=0,
                    ap=[[0, 8], [HW, S], [1, HW]],
                ),
            )

            # --- V path: xa = xt * e (two halves overlap the V matmuls);
            # contract (ci8,t) on PE; normalize on the PSUM read ---
            s_xa = sb.tile([128, 8, HW], bf16)
            p_o = pso.tile([C, HW], f32, tag="out")
            for halfg in range(2):
                nc.vector.tensor_tensor(
                    s_xa[:, 4 * halfg : 4 * halfg + 4, :],
                    s_xt[:, 4 * halfg : 4 * halfg + 4, :],
                    s_eb.unsqueeze(1).broadcast_to((128, 4, HW)),
                    op=ALU.mult,
                )
                for g in range(4 * halfg, 4 * halfg + 4):
                    nc.tensor.matmul(
                        p_o,
                        s_const[:, OFF_W1VR + C * g : OFF_W1VR + C * (g + 1)],
                        s_xa[:, g, :],
                        start=(g == 0),
                        stop=(g == 7),
                    )
            s_o = sb.tile([C, HW], f32)
            nc.vector.tensor_tensor(s_o, p_o, s_rcpb, op=ALU.mult)
            nc.sync.dma_start(o_d.ap(), s_o)

    nc.compile()
    return nc


def _get_module():
    global _MODULE
    if _MODULE is None:
        _MODULE = _build_module()
    return _MODULE


def make_host_inputs(x, w1, b1, w2, b2):
    """Host-side precompute: folded weights + per-core reflect-padded slices."""
    import ml_dtypes

    bf16 = ml_dtypes.bfloat16
    x = np.ascontiguousarray(np.asarray(x, np.float32))
    w1 = np.asarray(w1, np.float32)
    w2 = np.asarray(w2, np.float32)

    w1K = w1[C : 2 * C, :, 0, 0]          # [c, ci]
    w2K = w2[0, C : 2 * C]                # [c, 5, 5]
    weff = np.einsum("ci,cyx->iyx", w1K, w2K).reshape(C, NTAP)
    w1V = w1[2 * C :, :, 0, 0]            # [co, ci]

    # w1vr[(ci8,t), g, co] = w1V[co, 8g+ci8]
    tmp = w1V.T.reshape(8, 8, C)                      # (g, ci8, co)
    w1vr = (
        np.broadcast_to(tmp[:, :, None, :], (8, 8, S, C))
        .transpose(1, 2, 0, 3)
        .reshape(128, 8 * C)
    )

    # spatial partition packing is (h,t): p = h*16 + t
    hsel = np.zeros((128, ROWS), np.float32)
    for h in range(ROWS):
        hsel[h * S : (h + 1) * S, h] = 1.0
    irep = np.zeros((S, 128), np.float32)             # [t', (ci8,t)]
    for t in range(S):
        irep[t, t::S] = 1.0

    const = np.zeros((128, NCONST), np.float32)
    const[:C, OFF_WEFF : OFF_WEFF + NTAP] = weff
    const[:, OFF_HSEL : OFF_HSEL + ROWS] = hsel
    const[:S, OFF_IREP : OFF_IREP + 128] = irep
    const[:, OFF_W1VR:] = w1vr
    const = const.astype(bf16)

    in_maps = []
    for core in range(NCORES):
        b, hc = divmod(core, 4)
        h0 = ROWS * hc
        xp = np.pad(x[b], ((0, 0), (PAD, PAD), (PAD, PAD), (0, 0)), mode="reflect")
        slab = np.ascontiguousarray(xp[:, h0 : h0 + SLAB_R, :, :]).astype(bf16)
        xs = x[b][:, h0 : h0 + ROWS, :, :]            # [ci, h, w, t]
        xt = np.ascontiguousarray(
            xs.reshape(8, 8, ROWS, W, S)
            .transpose(1, 4, 0, 2, 3)
            .reshape(128, 8, HW)
        ).astype(bf16)
        in_maps.append({"slab": slab, "xt": xt, "cst": const})
    return in_maps


def assemble_output(results, b1):
    b1V = np.asarray(b1, np.float32)[2 * C :]
    out = np.empty((B, C, H, W, S), np.float32)
    for core in range(NCORES):
        b, hc = divmod(core, 4)
        h0 = ROWS * hc
        o = results[core]["o"].reshape(C, ROWS, W).astype(np.float32)
        out[b, :, h0 : h0 + ROWS, :, :] = (
            o[:, :, :, None] + b1V[:, None, None, None]
        )
    return out


def kernel(x, w1, b1, w2, b2):
    from concourse.bass_utils import run_bass_kernel_spmd

    nc = _get_module()
    in_maps = make_host_inputs(x, w1, b1, w2, b2)
    res = run_bass_kernel_spmd(nc, in_maps, core_ids=list(range(NCORES)))
    return assemble_output(res.results, b1)
